# revision 67
# baseline (speedup 1.0000x reference)
"""GQA attention block (q 32 heads / kv 8 heads, T=2048, C=4096) on 8 trn2
NeuronCores.

Sharding: tensor-parallel over heads x data-parallel over batch.
Core c handles batch b = c//4 and head-group g = c%4 (8 q heads, 2 kv heads).
Each core computes q/k/v projections for its head slice, RoPE, causal
flash-attention, and a row-parallel slice of the output projection; the host
sums the 4 partial outputs per batch.

All matmul operands are bf16 (quantization error ~6e-3 vs the 2e-2 gate;
PE streams bf16 at the same ~0.45 ns/col as fp32r, but halved SBUF/DMA lets
q^T and y^T live entirely in SBUF with no DRAM round trip).

Device-side layouts keep the head dim (hs) on SBUF partitions:
  Q^T, K^T, Y^T: [hs=128, tok]  (projections emit transposed directly; RoPE's
                  rotate-half runs as an SBUF partition-swap DMA with the
                  sign folded into the host sin table)
  V:   [tok, hs]  (projected directly in natural layout: lhsT=x chunk,
                   rhs=Wv -> no PE transpose)
  scores^T: [tk, tq]

Softmax denominators: exp'd probability chunks are accumulated in SBUF by the
vector engine; ONE ones-matmul per (head, 512-query block) turns the
accumulator into column sums (vs. one per 128-key chunk before: -123k PE
columns). The reciprocal + y-normalization (a ~4us DVE monolith) is split
into 128-column parts drip-fed between later attention chunks.

Schedule: P1 (K/V proj) -> P2 in two 4-head passes (wq half streamed per
pass; attention blocks interleaved into the projection stream as their q/k
become ready, and pumped across the pass boundary to hide the wq reload) ->
drain (leftover attention alternated with output-projection items reading
y^T straight from SBUF). The PE never waits on a phase boundary.
"""

import os
import sys

for _p in ("/root/.axon_site", "/root/.axon_site/_ro/trn_rl_repo",
           "/root/.axon_site/_ro/pypackages", "/opt/trn_rl_repo", "/opt/pypackages"):
    if os.path.isdir(_p) and _p not in sys.path:
        sys.path.append(_p)

import numpy as np
import ml_dtypes

import concourse.bass as bass
import concourse.tile as tile
from concourse import mybir
from concourse.bass_utils import run_bass_kernel_spmd

F32 = mybir.dt.float32
F32R = mybir.dt.float32r
BF16 = mybir.dt.bfloat16
NPBF16 = ml_dtypes.bfloat16

B, T, C = 2, 2048, 4096
H, KVH, HS = 32, 8, 128
ROPE_BASE = 10000.0

NCORES = 8
TPG = 4               # tensor-parallel groups per batch
HL = H // TPG         # 8 local q heads
KVL = KVH // TPG      # 2 local kv heads
GQ = H // KVH         # 4 q heads per kv head
CCH = C // 128        # 32 contraction chunks
PBLK = 256            # proj token block (free dim)
NPB = T // PBLK       # 8
TQB = 512             # attention tq block
NTQ = T // TQB        # 4
NTK = T // 128        # 16 tk chunks
NOC = C // TQB        # 8 output-proj column chunks
NCH = HL * HS // 128  # 8 output-proj contraction chunks
SCALE = float(1.0 / np.sqrt(HS))


def _split1(nc, max_waits=1):
    """Split instructions with >1 sem wait into preceding NOPs (the cayman
    CTRL codegen only accepts one sync-wait command per instruction)."""
    n = 0
    for f in nc.m.functions:
        for bb in f.blocks:
            out = []
            for inst in bb.instructions:
                si = inst.sync_info
                if si is not None and si.on_wait and len(si.on_wait) > max_waits:
                    w = list(si.on_wait)
                    chunks = [w[i:i + max_waits] for i in range(0, len(w), max_waits)]
                    for j, ch in enumerate(chunks[:-1]):
                        out.append(mybir.InstNoOp(
                            name=f"{inst.name}-wsplit{j}", engine=inst.engine,
                            ins=[], outs=[],
                            sync_info=mybir.SyncInfo(on_wait=ch, on_update=[])))
                        n += 1
                    inst.sync_info = mybir.SyncInfo(
                        on_wait=chunks[-1], on_update=list(si.on_update))
                out.append(inst)
            bb.instructions[:] = out
    return n


def build_nc(with_bias=False, split=True):
    nc = bass.Bass("TRN2")
    P = 128

    # --- DRAM parameters (per-core shards, host-pre-tiled layouts) ---
    dp = nc.declare_dram_parameter
    xq_d = dp("xq", [P, NPB, CCH, PBLK], BF16, isOutput=False)   # [ki, blk, ko, tw]
    xkv_d = dp("xkv", [P, NPB, CCH, PBLK], BF16, isOutput=False)
    # wq/wo are pre-split on the host so every DMA slice is contiguous per
    # partition: a strided slice costs thousands of 1KB descriptors, and the
    # descriptor generation occupies the issuing engine queue for ~10us.
    wq_d = dp("wq", [2, P, CCH, HL * HS // 2], BF16, isOutput=False)
    wkv_d = dp("wkv", [2, P, CCH, KVL * HS], BF16, isOutput=False)   # [K2 | V2]
    wo_d = dp("wo", [NOC, P, NCH, TQB], BF16, isOutput=False)
    cosT_d = dp("cosT", [HS, T], BF16, isOutput=False)
    sinT_d = dp("sinT", [HS, T], BF16, isOutput=False)           # sign-folded
    ones_d = dp("ones", [P, P], F32R, isOutput=False)
    mask_d = dp("mask", [P, 2 * TQB], BF16, isOutput=False)      # mult. causal
    if with_bias:
        bq_d = dp("bq", [HL * HS], F32, isOutput=False)
        bkv_d = dp("bkv", [2 * KVL * HS], F32, isOutput=False)
    out_d = dp("out", [T, C], BF16, isOutput=True)

    with tile.TileContext(nc) as tc:
        with (
            tc.tile_pool(name="consts", bufs=1) as consts,
            tc.tile_pool(name="kvres", bufs=1) as kvres,
            tc.tile_pool(name="qyres", bufs=1) as qyres,
            tc.tile_pool(name="pbuf", bufs=6) as pbuf,
            tc.tile_pool(name="accp", bufs=2) as accp,
            tc.tile_pool(name="rbuf", bufs=2) as rbuf,
            tc.tile_pool(name="pp_s", bufs=2, space="PSUM") as pp_s,
            tc.tile_pool(name="pp_y", bufs=2, space="PSUM") as pp_y,
            tc.tile_pool(name="pp_sum", bufs=1, space="PSUM") as pp_sum,
        ):
            ones_sb = consts.tile([P, P], F32R)
            cos_sb = consts.tile([HS, T], BF16)
            sin_sb = consts.tile([HS, T], BF16)
            mask_sb = consts.tile([P, 2 * TQB], BF16)
            kT_sb = kvres.tile([HS, KVL, T], BF16)
            v_sb = kvres.tile([P, NTK, KVL * HS], BF16)
            qT_sb = qyres.tile([HS, HL, T], BF16)
            yT_sb = qyres.tile([HS, HL, T], BF16)

            if with_bias:
                bq_sb = consts.tile([P, HL], F32)
                bkv_sb = consts.tile([P, 2 * KVL], F32)
                bvrow_sb = consts.tile([1, KVL * HS], BF16)
                ones1_sb = consts.tile([1, P], BF16)
                nc.scalar.dma_start(out=bq_sb, in_=bq_d[:].rearrange("(h p) -> p h", p=P))
                nc.scalar.dma_start(out=bkv_sb, in_=bkv_d[:].rearrange("(h p) -> p h", p=P))
                nc.scalar.dma_start(
                    out=bvrow_sb, in_=bkv_d[KVL * HS:].rearrange("n -> 1 n"))
                nc.vector.memset(ones1_sb, 1.0)

            # ---------------- attention block ----------------
            # The softmax finalization (reciprocal + y normalization) is a
            # ~4us DVE monolith; emitted inline it blocks the next block's
            # mask/accumulate chain (and with it the exps, via pT buffer
            # recycling) and the PE stalls once per block. Instead it is
            # split into 128-column parts queued on `fin_q` and drip-fed
            # between the chunks of subsequent attention blocks.
            # Output-projection matmuls are dripped one-per-chunk into the
            # attention stream (p4_feed): the attention chain is exp-bound
            # (~690ns/chunk on scalar vs ~460ns of PE work), and the PE queue
            # is in-order, so only independent matmuls emitted BETWEEN the
            # dependent chunk pairs can fill the per-chunk PE bubbles.
            fin_q = []
            p4_feed = []
            blk_seq = [0]

            def attn_block(h, j):
                # parts two generations old touch buffers this block's ring
                # slot is about to reuse -- they must be emitted first
                seq = blk_seq[0]
                blk_seq[0] += 1
                while fin_q and fin_q[0][0] <= seq - 2:
                    fin_q.pop(0)[1]()
                kv = h // GQ
                qb = qT_sb[:, h, j * TQB:(j + 1) * TQB]
                ps_y = pp_y.tile([P, TQB], F32, tag="y")
                acc = accp.tile([P, TQB], F32R, tag="acc")
                nchunks = (j + 1) * (TQB // P)

                def flush(pend, last):
                    pT0, a0, off0, w0 = pend
                    nc.tensor.matmul(
                        ps_y[:, off0:],
                        lhsT=v_sb[:, a0, kv * HS:(kv + 1) * HS],
                        rhs=pT0[:, :w0], start=(a0 == 0), stop=last)

                pend = None
                for a in range(nchunks):
                    m = a - j * (TQB // P)
                    off = P * m if m > 0 else 0
                    w = TQB - off
                    ps_s = pp_s.tile([P, TQB], F32, tag="s")
                    nc.tensor.matmul(
                        ps_s[:, :w], lhsT=kT_sb[:, kv, a * P:(a + 1) * P],
                        rhs=qb[:, off:], start=True, stop=True)
                    if pend is not None:
                        flush(pend, last=False)
                    pT = pbuf.tile([P, TQB], BF16, tag="pT")
                    nc.scalar.activation(
                        out=pT[:, :w], in_=ps_s[:, :w],
                        func=mybir.ActivationFunctionType.Exp,
                        scale=SCALE)
                    if m >= 0:                 # diagonal: triangle mask
                        nc.vector.tensor_mul(
                            pT[:, :w], pT[:, :w],
                            mask_sb[:, TQB: TQB + w])
                    if a == 0:
                        nc.vector.tensor_copy(out=acc, in_=pT)
                    else:
                        nc.vector.tensor_add(
                            acc[:, off:], acc[:, off:], pT[:, :w])
                    if fin_q:
                        fin_q.pop(0)[1]()      # drip one finalize part
                    if p4_feed:
                        p4_feed.pop(0)()       # drip one out-proj matmul
                    pend = (pT, a, off, w)
                flush(pend, last=True)
                ps_sum = pp_sum.tile([P, TQB], F32, tag="sum")
                nc.tensor.matmul(ps_sum, lhsT=ones_sb, rhs=acc,
                                 start=True, stop=True)
                rec = rbuf.tile([P, TQB], F32, tag="rec")
                yo = yT_sb[:, h, j * TQB:(j + 1) * TQB]
                for q in range(4):
                    sl = slice(q * P, (q + 1) * P)
                    fin_q.append((seq, lambda sl=sl, rec=rec, ps_sum=ps_sum:
                                  nc.vector.reciprocal(
                                      out=rec[:, sl], in_=ps_sum[:, sl])))
                for q in range(4):
                    sl = slice(q * P, (q + 1) * P)
                    fin_q.append((seq, lambda sl=sl, rec=rec, ps_y=ps_y, yo=yo:
                                  nc.vector.tensor_mul(
                                      yo[:, sl], ps_y[:, sl], rec[:, sl])))

            def fin_flush():
                while fin_q:
                    fin_q.pop(0)[1]()

            # ---------- P1 + P2 (projection streams, attn interleaved) ----------
            attnq = ([(h, j) for j in range(NTQ) for h in range(HL // 2)]
                     + [(h, j) for j in range(NTQ) for h in range(HL // 2, HL)])
            attn_done = set()

            with (
                tc.tile_pool(name="xpool", bufs=4) as xpool,
                tc.tile_pool(name="rotp", bufs=2) as rotp,
                tc.tile_pool(name="ptmp", bufs=2) as ptmp,
                tc.tile_pool(name="wkvp", bufs=1) as wkvp,
                tc.tile_pool(name="wqp", bufs=1) as wqp,
                tc.tile_pool(name="pp_proj", bufs=3, space="PSUM") as pp_proj,
            ):
                def rope(dst, ps, blk, bias_col=None):
                    """dst = rope(ps + bias) over a [128, PBLK] block.

                    rotate-half = SBUF partition-swap DMA (sign folded into
                    the host sin table); muls on DVE in fp32, one bf16
                    rounding on the final add into dst.
                    """
                    sl = slice(blk * PBLK, (blk + 1) * PBLK)
                    src = ptmp.tile([P, PBLK], F32, tag="raw")
                    if bias_col is not None:
                        nc.vector.tensor_scalar(
                            out=src, in0=ps, scalar1=bias_col, scalar2=None,
                            op0=mybir.AluOpType.add)
                    else:
                        nc.vector.tensor_copy(out=src, in_=ps)
                    rot = rotp.tile([P, PBLK], F32, tag="rot")
                    nc.sync.dma_start(out=rot[0:64, :], in_=src[64:128, :])
                    nc.sync.dma_start(out=rot[64:128, :], in_=src[0:64, :])
                    tmp = ptmp.tile([P, PBLK], F32, tag="tmp")
                    nc.vector.tensor_mul(tmp, rot, sin_sb[:, sl])
                    nc.vector.tensor_mul(src, src, cos_sb[:, sl])
                    nc.vector.tensor_add(dst, src, tmp)

                def load_x_quarters(x_d, blk):
                    # halves split between the sync and gpsimd DMA queues:
                    # one queue can't keep up with a ~17us block while also
                    # carrying the rope swaps
                    xa = xpool.tile([P, CCH // 2, PBLK], BF16, tag="x")
                    xb = xpool.tile([P, CCH // 2, PBLK], BF16, tag="x")
                    nc.sync.dma_start(out=xa, in_=x_d[:, blk, :CCH // 2])
                    nc.gpsimd.dma_start(out=xb, in_=x_d[:, blk, CCH // 2:])
                    return [xa, xb]

                def proj_group(ps, w_t, xq, ch):
                    for c in range(CCH):
                        nc.tensor.matmul(
                            ps, lhsT=w_t[:, c, ch * P:(ch + 1) * P],
                            rhs=xq[c // (CCH // 2)][:, c % (CCH // 2), :],
                            start=(c == 0), stop=(c == CCH - 1))

                # weights on the gpsimd DMA queue: the scalar queue carries
                # the attention exps, and a WAR-gated weight trigger there
                # blocks them (and with them the PE) for the whole transfer
                wk_t = wkvp.tile([P, CCH, KVL * HS], BF16, tag="wk")
                wv_t = wkvp.tile([P, CCH, KVL * HS], BF16, tag="wv")
                nc.gpsimd.dma_start(out=wk_t[:, :CCH // 2], in_=wkv_d[0, :, :CCH // 2])
                nc.gpsimd.dma_start(out=wk_t[:, CCH // 2:], in_=wkv_d[0, :, CCH // 2:])
                nc.scalar.dma_start(out=cos_sb, in_=cosT_d[:])
                nc.scalar.dma_start(out=sin_sb, in_=sinT_d[:])
                nc.scalar.dma_start(out=mask_sb, in_=mask_d[:])
                nc.scalar.dma_start(out=ones_sb, in_=ones_d[:])

                # ---- P1: K^T (rope) + V (natural layout, no transpose) ----
                # x blocks are prefetched one block ahead (incl. across the
                # P1->P2 and pass boundaries): a late x DMA doesn't just stall
                # the PE, it drops the HAM clock-gate to half rate for ~3.4us.
                nxt = load_x_quarters(xkv_d, 0)
                nc.gpsimd.dma_start(out=wv_t, in_=wkv_d[1])  # after x(0)
                for blk in range(NPB):
                    xq = nxt
                    nxt = (load_x_quarters(xkv_d, blk + 1) if blk + 1 < NPB
                           else load_x_quarters(xq_d, 0))
                    for ch in range(KVL):
                        ps = pp_proj.tile([P, PBLK], F32, tag="proj")
                        proj_group(ps, wk_t, xq, ch)
                        rope(kT_sb[:, ch, blk * PBLK:(blk + 1) * PBLK], ps, blk,
                             bkv_sb[:, ch:ch + 1] if with_bias else None)
                    for sub in range(PBLK // P):
                        psv = pp_proj.tile([P, PBLK], F32, tag="proj")
                        for c in range(CCH):
                            xt = xq[c // (CCH // 2)]
                            nc.tensor.matmul(
                                psv,
                                lhsT=xt[:, c % (CCH // 2), sub * P:(sub + 1) * P],
                                rhs=wv_t[:, c, :],
                                start=(c == 0),
                                stop=(c == CCH - 1 and not with_bias))
                        if with_bias:          # rank-1: ones^T @ bias row
                            nc.tensor.matmul(
                                psv, lhsT=ones1_sb, rhs=bvrow_sb,
                                start=False, stop=True)
                        nc.scalar.activation(
                            out=v_sb[:, (PBLK // P) * blk + sub, :], in_=psv,
                            func=mybir.ActivationFunctionType.Copy)

                # ---- P2: Q^T in two 4-head passes, attn interleaved ----
                def pump(half, blk, hb, budget=1, reserve=0):
                    emitted = 0
                    while emitted < budget and attnq:
                        if len(attn_done) >= 32 - reserve:
                            break
                        h, j = attnq[0]
                        if h // (HL // 2) < half or (
                                h // (HL // 2) == half
                                and (blk, hb) >= (2 * j + 1, h % (HL // 2))):
                            attnq.pop(0)
                            attn_block(h, j)
                            attn_done.add((h, j))
                            emitted += 1
                        else:
                            break
                    return emitted

                for half in range(2):
                    wq_t = wqp.tile([P, CCH, HL * HS // 2], BF16, tag="wq")
                    for q in range(4):   # quarters across both DMA queues
                        sl = slice(q * (CCH // 4), (q + 1) * (CCH // 4))
                        eng = nc.gpsimd if q % 2 == 0 else nc.sync
                        eng.dma_start(out=wq_t[:, sl], in_=wq_d[half, :, sl])
                    if half == 1:      # cover the wq reload with reserved attn
                        pump(half, -1, 0, budget=4)
                    for blk in range(NPB):
                        xq = nxt
                        if blk + 1 < NPB:
                            nxt = load_x_quarters(xq_d, blk + 1)
                        elif half == 0:
                            nxt = load_x_quarters(xq_d, 0)  # pass-1 re-read
                        for hb in range(HL // 2):
                            h = half * (HL // 2) + hb
                            ps = pp_proj.tile([P, PBLK], F32, tag="proj")
                            proj_group(ps, wq_t, xq, hb)
                            rope(qT_sb[:, h, blk * PBLK:(blk + 1) * PBLK],
                                 ps, blk,
                                 bq_sb[:, h:h + 1] if with_bias else None)
                            # pass 0 pops 12 blocks (4 held back to cover the
                            # wq reload at the boundary); pass 1 pops nothing:
                            # heads 4-7 run in the drain interleaved with the
                            # out-projection, where their exp-chain bubbles
                            # are filled by p4 matmuls
                            if half == 0:
                                pump(half, blk, hb, reserve=20)

            # ---------- drain: leftover attn alternated with out-proj ----------
            with (
                tc.tile_pool(name="wop", bufs=1) as wop,
                tc.tile_pool(name="outb", bufs=3) as outb,
                tc.tile_pool(name="pp_o", bufs=3, space="PSUM") as pp_o,
            ):
                wo_t = wop.tile([P, NOC, NCH, TQB], BF16, tag="wo")
                for o in range(NOC):       # chunked so p4(j,0) starts early
                    nc.gpsimd.dma_start(out=wo_t[:, o], in_=wo_d[o])

                def p4_tile_thunks(o, i):
                    # one out-proj i-tile as 8 matmul thunks + an eviction
                    # thunk, so it can be dripped between attention chunks
                    st = {}

                    def mk_mm(chl):
                        def t():
                            if chl == 0:
                                ps_o = pp_o.tile([P, TQB], F32, tag="o")
                                st['ps'] = ps_o
                            nc.tensor.matmul(
                                st['ps'],
                                lhsT=yT_sb[:, chl, i * P:(i + 1) * P],
                                rhs=wo_t[:, o, chl, :],
                                start=(chl == 0), stop=(chl == NCH - 1))
                        return t

                    def evict():
                        ot = outb.tile([P, TQB], BF16, tag="ot")
                        nc.scalar.activation(
                            out=ot, in_=st['ps'],
                            func=mybir.ActivationFunctionType.Copy)
                        nc.sync.dma_start(
                            out=out_d[i * P:(i + 1) * P,
                                      o * TQB:(o + 1) * TQB],
                            in_=ot)
                    return [mk_mm(chl) for chl in range(NCH)] + [evict]

                p4q = [(j, o) for j in range(NTQ) for o in range(NOC)]

                def refill_feeder():
                    while p4q and all((h2, p4q[0][0]) in attn_done
                                      for h2 in range(HL)):
                        j, o = p4q.pop(0)
                        fin_flush()   # y^T for batch j must be fully written
                        for i in range(j * (TQB // P), (j + 1) * (TQB // P)):
                            p4_feed.extend(p4_tile_thunks(o, i))

                while attnq:
                    refill_feeder()
                    h, j = attnq.pop(0)
                    attn_block(h, j)   # drips p4_feed between chunks
                    attn_done.add((h, j))
                refill_feeder()
                fin_flush()
                while p4_feed:
                    p4_feed.pop(0)()

    if split:
        _split1(nc)
    return nc


def _rope_tables():
    inv_freq = (1.0 / (np.float32(ROPE_BASE) **
                       (np.arange(0, HS, 2, dtype=np.float32) / np.float32(HS))))
    pos = np.arange(T, dtype=np.float32)
    ang = pos[:, None] * inv_freq[None, :]
    ang = np.concatenate([ang, ang], axis=-1).astype(np.float32)  # [T, HS]
    return np.cos(ang).astype(np.float32), np.sin(ang).astype(np.float32)


def _tile_x(x):
    # [T, C] -> [ki=128, blk, ko, tw]  (x^T tiled for contraction-major DMA)
    return np.ascontiguousarray(
        x.reshape(NPB, PBLK, CCH, 128).transpose(3, 0, 2, 1)).astype(NPBF16)


def _tile_w(w):
    # [C, N] -> [ki=128, ko, N]
    n = w.shape[1]
    return np.ascontiguousarray(
        w.reshape(CCH, 128, n).transpose(1, 0, 2)).astype(NPBF16)


def _consts():
    ones = np.ones((128, 128), np.float32)
    u = np.arange(2 * TQB)[None, :]
    i = np.arange(128)[:, None]
    mask = (u >= i + TQB).astype(NPBF16)
    return ones, mask


_NC_CACHE = {}


def make_in_maps(inp, with_bias):
    q_x, kv_x = inp["q_x"], inp["kv_x"]
    Wq, Wk, Wv, Wo = inp["Wq"], inp["Wk"], inp["Wv"], inp["Wo"]
    cos, sin = _rope_tables()
    cosT = np.ascontiguousarray(cos.T).astype(NPBF16)   # [HS, T]
    sinT = np.ascontiguousarray(sin.T).copy()
    sinT[:64, :] *= -1.0                                # sign of rotate-half
    sinT = sinT.astype(NPBF16)
    ones, mask = _consts()
    xq_tiles = {b: _tile_x(q_x[b]) for b in range(B)}
    xkv_tiles = {b: _tile_x(kv_x[b]) for b in range(B)}
    in_maps = []
    for core in range(NCORES):
        b, g = core // TPG, core % TPG
        m = {
            "xq": xq_tiles[b],
            "xkv": xkv_tiles[b],
            "wq": np.ascontiguousarray(np.stack([
                _tile_w(Wq[:, g * HL * HS:(g + 1) * HL * HS])[:, :, :512],
                _tile_w(Wq[:, g * HL * HS:(g + 1) * HL * HS])[:, :, 512:]])),
            "wkv": np.ascontiguousarray(np.stack(
                [_tile_w(Wk[:, g * KVL * HS:(g + 1) * KVL * HS]),
                 _tile_w(Wv[:, g * KVL * HS:(g + 1) * KVL * HS])])),
            # [o, ki, chl, tw] so each o-chunk DMA is contiguous
            "wo": np.ascontiguousarray(
                Wo[g * HL * HS:(g + 1) * HL * HS, :]
                .reshape(NCH, 128, NOC, TQB).transpose(2, 1, 0, 3)).astype(NPBF16),
            "cosT": cosT, "sinT": sinT,
            "ones": ones, "mask": mask,
        }
        if with_bias:
            m["bq"] = np.ascontiguousarray(inp["bq"][g * HL * HS:(g + 1) * HL * HS])
            m["bkv"] = np.concatenate(
                [inp["bk"][g * KVL * HS:(g + 1) * KVL * HS],
                 inp["bv"][g * KVL * HS:(g + 1) * KVL * HS]])
        in_maps.append(m)
    return in_maps


def kernel(**inputs):
    inp = {k: np.asarray(v, dtype=np.float32) for k, v in inputs.items()}
    with_bias = bool(np.any(inp["bq"]) or np.any(inp["bk"]) or np.any(inp["bv"]))

    if ("nc", with_bias) not in _NC_CACHE:
        _NC_CACHE[("nc", with_bias)] = build_nc(with_bias=with_bias)
    nc = _NC_CACHE[("nc", with_bias)]

    in_maps = make_in_maps(inp, with_bias)
    res = run_bass_kernel_spmd(nc, in_maps, list(range(NCORES)))
    out = np.zeros((B, T, C), np.float32)
    for core in range(NCORES):
        out[core // TPG] += res.results[core]["out"].astype(np.float32)
    out += inp["bo"]
    return out
